# revision 18
# baseline (speedup 1.0000x reference)
"""AFNO transformer block (LN -> AFNO2D -> +res -> LN -> MLP -> +res) on 8 trn2 cores.

Distribution: spatial (b,h)-rows sharded 8x for LN1/FFT-W and iFFT-W/LN2/MLP stages;
kw-spectral-columns sharded 8x for FFT-H/block-MM/iFFT-H middle stage; two
AllToAlls (bf16 spectra) swap between the shardings. FFTs are dense matmuls
against DFT matrices (only 46 of 91 W-modes survive; all H modes kept).

v2: LN work spread across Scalar/GpSimd/Vector engines; LN2 folded into fc1
weights host-side; residuals stored bf16; softshrink via two Relu activations;
stage-B LN/iFFT interleaved with MLP chunks so Vector work hides under the
Tensor-bound matmuls.
"""
import sys

sys.path.insert(0, "/opt/trn_rl_repo")
import numpy as np
from ml_dtypes import bfloat16

from concourse import bacc, tile, mybir
from concourse import bass_utils
from concourse.masks import make_identity

FP = mybir.dt.float32
BF = mybir.dt.bfloat16
AF = mybir.ActivationFunctionType
ALU = mybir.AluOpType

NCORES = 8
B, H, W, C = 2, 90, 180, 768
NB, BS = 8, 96
KW, KWP = 46, 48
ROWS = B * H            # 180
RPAD = 192              # padded rows
RPC = RPAD // NCORES    # 24 rows per core
KWL = KWP // NCORES     # 6 kw per core
HID = 4 * C             # 3072
LN_EPS = 1e-5
LAM = 0.01              # softshrink lambda
TOK = RPC * W           # 4320 tokens per core in stage B
WCH = [(0, 128), (128, 52)]   # w-axis partition chunks
CS = [(0, 384), (384, 384)]   # channel free-dim slices

_cache = {}


def _dft_consts():
    wi, ki, hi = np.arange(W), np.arange(KWP), np.arange(H)
    aw = 2 * np.pi * np.outer(wi, ki) / W
    FWr = np.cos(aw) / np.sqrt(W)
    FWi = -np.sin(aw) / np.sqrt(W)
    FWr[:, KW:] = 0.0
    FWi[:, KW:] = 0.0
    fw = np.concatenate([FWr, FWi], axis=1)          # [180, 96]
    ah = 2 * np.pi * np.outer(hi, hi) / H
    ehr = np.cos(ah) / np.sqrt(H)                    # symmetric
    ehi = -np.sin(ah) / np.sqrt(H)
    ck = np.where(ki == 0, 1.0, 2.0)
    CWr = (ck[:, None] * np.cos(aw.T)) / np.sqrt(W)  # [48, 180]
    CWi = (-ck[:, None] * np.sin(aw.T)) / np.sqrt(W)
    CWi[0, :] = 0.0
    CWr[KW:, :] = 0.0
    CWi[KW:, :] = 0.0
    cw = np.concatenate([CWr, CWi], axis=0)          # [96, 180]
    # interleave real/imag so spectra rows come out in (kw, comp) order,
    # matching the a2a tensors' [s, kwl, comp] layout -> single-DMA moves
    il = np.array([[k, KWP + k] for k in range(KWP)]).reshape(-1)
    fw = fw[:, il]
    cw = cw[il, :]
    b16 = lambda a: np.ascontiguousarray(a).astype(bfloat16)
    padf = lambda a: np.concatenate([a, np.zeros((a.shape[0], 128 - a.shape[1]),
                                                 a.dtype)], axis=1)
    return dict(fw=b16(fw), ehr=b16(ehr), ehi=b16(ehi), ehin=b16(-ehi),
                eir=b16(padf(ehr)), eii=b16(padf(-ehi)), eiin=b16(padf(ehi)),
                cw=b16(cw))


def _build(fast1):
    nc = bacc.Bacc("TRN2", target_bir_lowering=False, debug=False,
                   num_devices=NCORES)

    def din(name, shape, dt=FP):
        return nc.dram_tensor(name, list(shape), dt, kind="ExternalInput").ap()

    x_sh = din("x_sh", [RPC, W, C])
    fw_d = din("fw_d", [W, 2 * KWP], BF)
    ehr_d = din("ehr_d", [H, H], BF)
    ehi_d = din("ehi_d", [H, H], BF)
    ehin_d = din("ehin_d", [H, H], BF)
    eir_d = din("eir_d", [H, 128], BF)
    eii_d = din("eii_d", [H, 128], BF)
    eiin_d = din("eiin_d", [H, 128], BF)
    cw_d = din("cw_d", [2 * KWP, W], BF)
    blk1_d = din("blk1_d", [NB, 3, BS, 128], BF)  # [w1r, w1i, -w1i], out-dim padded
    blk2_d = din("blk2_d", [NB, 3, BS, 128], BF)
    b1_d = din("b1_d", [2, NB, BS])
    b2m_d = din("b2m_d", [2, NB, BS])      # b2 - lam
    b2pn_d = din("b2pn_d", [2, NB, BS])    # -(b2 + lam)
    fc1w_d = din("fc1w_d", [C, HID], BF)   # LN2 weight folded in
    fc2w_d = din("fc2w_d", [HID, C], BF)
    fc1b_d = din("fc1b_d", [1, HID])       # LN2 bias folded in
    fc2b_d = din("fc2b_d", [1, C])
    if not fast1:
        n1w_d = din("n1w_d", [1, C])
        n1b_d = din("n1b_d", [1, C])

    out_sh = nc.dram_tensor("out_sh", [RPC, W, C], FP, kind="ExternalOutput").ap()

    s_dram = nc.dram_tensor("s_dram", [RPC, W, C], BF).ap()
    h_dram = nc.dram_tensor("h_dram", [RPC, W, C], BF).ap()
    a2a1_in = nc.dram_tensor("a2a1_in", [NCORES, RPC, KWL, 2, C], BF).ap()
    a2a1_out = nc.dram_tensor("a2a1_out", [NCORES, RPC, KWL, 2, C], BF).ap()
    a2a2_in = nc.dram_tensor("a2a2_in", [NCORES, RPC, KWL, 2, C], BF).ap()
    a2a2_out = nc.dram_tensor("a2a2_out", [NCORES, RPC, KWL, 2, C], BF).ap()

    rg = [list(range(NCORES))]

    with tile.TileContext(nc) as tc:
        with tc.tile_pool(name="cpool", bufs=1) as cp:
            # ---- small constants resident whole kernel ----
            ident = cp.tile([128, 128], BF)
            make_identity(nc, ident[:])
            fw_a = cp.tile([128, 2 * KWP], BF)
            fw_b = cp.tile([52, 2 * KWP], BF)
            nc.sync.dma_start(out=fw_a[:], in_=fw_d[0:128, :])
            nc.sync.dma_start(out=fw_b[:], in_=fw_d[128:180, :])
            ehr = cp.tile([H, H], BF); nc.sync.dma_start(out=ehr[:], in_=ehr_d[:])
            ehi = cp.tile([H, H], BF); nc.sync.dma_start(out=ehi[:], in_=ehi_d[:])
            ehin = cp.tile([H, H], BF); nc.sync.dma_start(out=ehin[:], in_=ehin_d[:])
            eir = cp.tile([H, 128], BF); nc.sync.dma_start(out=eir[:], in_=eir_d[:])
            eii = cp.tile([H, 128], BF); nc.sync.dma_start(out=eii[:], in_=eii_d[:])
            eiin = cp.tile([H, 128], BF); nc.sync.dma_start(out=eiin[:], in_=eiin_d[:])
            cw = cp.tile([2 * KWP, W], BF); nc.sync.dma_start(out=cw[:], in_=cw_d[:])
            b1c = cp.tile([BS, 2, NB], FP)
            b2m = cp.tile([BS, 2, NB], FP)
            b2pn = cp.tile([BS, 2, NB], FP)
            nc.sync.dma_start(out=b1c[:], in_=b1_d.rearrange("v k o -> o v k"))
            nc.sync.dma_start(out=b2m[:], in_=b2m_d.rearrange("v k o -> o v k"))
            nc.sync.dma_start(out=b2pn[:], in_=b2pn_d.rearrange("v k o -> o v k"))
            epsb = cp.tile([128, 1], FP)
            nc.vector.memset(epsb[:], LN_EPS)
            fc1b = cp.tile([128, 24], FP)
            fc2b = cp.tile([128, 6], FP)
            nc.sync.dma_start(out=fc1b[:], in_=fc1b_d.rearrange("x (m p) -> p (x m)", p=128))
            nc.sync.dma_start(out=fc2b[:], in_=fc2b_d.rearrange("x (m p) -> p (x m)", p=128))

            # ================= STAGE A: LN1 + FFT-W per row =================
            with (
                tc.tile_pool(name="sa", bufs=6) as sa,
                tc.tile_pool(name="na", bufs=1) as na,
                tc.tile_pool(name="saps", bufs=2, space="PSUM") as saps,
            ):
                if not fast1:
                    n1w_r = na.tile([128, 2, 384], FP)
                    n1b_r = na.tile([128, 2, 384], FP)
                    nc.sync.dma_start(out=n1w_r[:], in_=n1w_d[:].partition_broadcast(128))
                    nc.sync.dma_start(out=n1b_r[:], in_=n1b_d[:].partition_broadcast(128))
                for r in range(RPC):
                    h1bf = []
                    for ci, (w0, wn) in enumerate(WCH):
                        xa = sa.tile([128, 2, 384], FP, tag=f"xa{ci}", name=f"xa_{r}_{ci}")
                        nc.sync.dma_start(out=xa[:wn], in_=x_sh[r, w0:w0 + wn, :])
                        xf = sa.tile([128, 2, 384], BF, tag=f"xf{ci}", name=f"xf_{r}_{ci}")
                        if ci == 0:
                            nc.gpsimd.tensor_copy(xf[:wn], xa[:wn])
                        else:
                            nc.vector.tensor_copy(xf[:wn], xa[:wn])
                        st = sa.tile([128, 2, 6], FP, tag=f"st{ci}", name=f"st_{r}_{ci}")
                        nc.vector.bn_stats(st[:wn, 0, :], xf[:wn, 0])
                        nc.vector.bn_stats(st[:wn, 1, :], xf[:wn, 1])
                        ag = sa.tile([128, 2], FP, tag=f"ag{ci}", name=f"ag_{r}_{ci}")
                        nc.vector.bn_aggr(ag[:wn], st[:wn])
                        sd = sa.tile([128, 1], FP, tag=f"sd{ci}", name=f"sd_{r}_{ci}")
                        nc.scalar.activation(sd[:wn], ag[:wn, 1:2], AF.Sqrt,
                                             bias=epsb[:wn])
                        rs = sa.tile([128, 1], FP, tag=f"rs{ci}", name=f"rs_{r}_{ci}")
                        nc.vector.reciprocal(rs[:wn], sd[:wn])
                        tb_ = sa.tile([128, 2, 384], BF, tag=f"tb{ci}", name=f"tb_{r}_{ci}")
                        nc.vector.tensor_scalar(tb_[:wn], xf[:wn], ag[:wn, 0:1],
                                                rs[:wn], ALU.subtract, ALU.mult)
                        if fast1:
                            hbf = tb_
                        else:
                            hw = sa.tile([128, 2, 384], BF, tag=f"hw{ci}", name=f"hw_{r}_{ci}")
                            nc.gpsimd.tensor_mul(hw[:wn], tb_[:wn], n1w_r[:wn])
                            hbf = sa.tile([128, 2, 384], BF, tag=f"hb{ci}", name=f"hb_{r}_{ci}")
                            nc.gpsimd.tensor_add(hbf[:wn], hw[:wn], n1b_r[:wn])
                        # s_dram holds LN1(x) only; stage B re-adds x
                        nc.scalar.dma_start(out=s_dram[r, w0:w0 + wn, :], in_=hbf[:wn])
                        h1bf.append(hbf)
                    for si, (c0, cn) in enumerate(CS):
                        psy = saps.tile([2 * KWP, 384], FP, tag="psy", name=f"psy_{r}_{si}")
                        nc.tensor.matmul(psy[:], fw_a[:], h1bf[0][:, si],
                                         start=True, stop=False)
                        nc.tensor.matmul(psy[:], fw_b[:], h1bf[1][0:52, si],
                                         start=False, stop=True)
                        yb = sa.tile([2 * KWP, 384], BF, tag="yb", name=f"yb_{r}_{si}")
                        nc.scalar.copy(yb[:], psy[:])
                        nc.sync.dma_start(out=a2a1_in[:, r, :, :, c0:c0 + cn],
                                          in_=yb[:])

            nc.gpsimd.collective_compute(
                "AllToAll", ALU.bypass, replica_groups=rg,
                ins=[a2a1_in[:]], outs=[a2a1_out[:]])

            # ================= MIDDLE: FFT-H, block MM, iFFT-H =================
            v1 = a2a1_out.rearrange("s r kwl comp c -> (s r) kwl comp c")
            v2in = a2a2_in.rearrange("s r kwl comp c -> (s r) kwl comp c")
            NTOK = B * KWL * H  # 1080 spectral tokens per core
            with tc.tile_pool(name="mz", bufs=1) as mz:
                zsb = mz.tile([BS, NB, 2, NTOK], BF)   # [96, k, comp, (b kwl kh)]
                osb = mz.tile([H, B, KWL, 2, NB, BS], BF)
                blk1 = mz.tile([BS, NB, 3, 128], BF)
                blk2 = mz.tile([BS, NB, 3, 128], BF)
                nc.sync.dma_start(out=blk1[:], in_=blk1_d.rearrange("k v i o -> i k v o"))
                nc.sync.dma_start(out=blk2[:], in_=blk2_d.rearrange("k v i o -> i k v o"))
                with (
                    tc.tile_pool(name="m1", bufs=2) as m1p,
                    tc.tile_pool(name="m1ps", bufs=2, space="PSUM") as m1ps,
                ):
                    for b in range(B):
                        ybt = m1p.tile([H, KWL, 2, C], BF, tag="ybt", name=f"ybt_{b}")
                        nc.sync.dma_start(out=ybt[:], in_=v1[b * H:(b + 1) * H])
                        for kwl in range(KWL):
                            for si, (c0, cn) in enumerate(CS):
                                pr = m1ps.tile([H, 384], FP, tag="pr", name=f"pr_{b}_{kwl}_{si}")
                                pi = m1ps.tile([H, 384], FP, tag="pi", name=f"pi_{b}_{kwl}_{si}")
                                rr = ybt[:, kwl, 0, c0:c0 + cn]
                                ri = ybt[:, kwl, 1, c0:c0 + cn]
                                nc.tensor.matmul(pr[:], ehr[:], rr, start=True, stop=False)
                                nc.tensor.matmul(pr[:], ehin[:], ri, start=False, stop=True)
                                nc.tensor.matmul(pi[:], ehi[:], rr, start=True, stop=False)
                                nc.tensor.matmul(pi[:], ehr[:], ri, start=False, stop=True)
                                tsb = m1p.tile([H, 2, 384], BF, tag="tsb",
                                               name=f"tsb_{b}_{kwl}_{si}")
                                nc.vector.tensor_copy(tsb[:, 0], pr[:])
                                nc.vector.tensor_copy(tsb[:, 1], pi[:])
                                for cb in range(4):
                                    k = si * 4 + cb
                                    pz = m1ps.tile([BS, 2, H], BF, tag="pz",
                                                   name=f"pz_{b}_{kwl}_{si}_{cb}")
                                    for comp in range(2):
                                        nc.tensor.transpose(
                                            pz[:, comp, :],
                                            tsb[:, comp, cb * BS:(cb + 1) * BS],
                                            ident[0:H, 0:H])
                                    tk0 = (b * KWL + kwl) * H
                                    if k % 2 == 0:
                                        nc.vector.tensor_copy(
                                            zsb[:, k, :, tk0:tk0 + H], pz[:])
                                    else:
                                        nc.scalar.copy(
                                            zsb[:, k, :, tk0:tk0 + H], pz[:])
                with (
                    tc.tile_pool(name="m2", bufs=2) as m2p,
                    tc.tile_pool(name="m2ps", bufs=2, space="PSUM") as m2ps,
                ):
                    BCH = [(0, 512), (512, 512), (1024, NTOK - 1024)]
                    for k in range(NB):
                        for t0, tn in BCH:
                            p1r = m2ps.tile([128, 512], FP, tag="p1r", name=f"p1r_{k}_{t0}")
                            p1i = m2ps.tile([128, 512], FP, tag="p1i", name=f"p1i_{k}_{t0}")
                            zr = zsb[:, k, 0, t0:t0 + tn]
                            zi = zsb[:, k, 1, t0:t0 + tn]
                            nc.tensor.matmul(p1r[:, :tn], blk1[:, k, 0], zr, start=True, stop=False)
                            nc.tensor.matmul(p1r[:, :tn], blk1[:, k, 2], zi, start=False, stop=True)
                            nc.tensor.matmul(p1i[:, :tn], blk1[:, k, 1], zr, start=True, stop=False)
                            nc.tensor.matmul(p1i[:, :tn], blk1[:, k, 0], zi, start=False, stop=True)
                            o1r = m2p.tile([BS, 512], BF, tag="o1r", name=f"o1r_{k}_{t0}")
                            o1i = m2p.tile([BS, 512], BF, tag="o1i", name=f"o1i_{k}_{t0}")
                            nc.scalar.activation(o1r[:, :tn], p1r[:BS, :tn], AF.Relu,
                                                 bias=b1c[:, 0, k:k + 1])
                            nc.scalar.activation(o1i[:, :tn], p1i[:BS, :tn], AF.Relu,
                                                 bias=b1c[:, 1, k:k + 1])
                            p2r = m2ps.tile([128, 512], FP, tag="p2r", name=f"p2r_{k}_{t0}")
                            p2i = m2ps.tile([128, 512], FP, tag="p2i", name=f"p2i_{k}_{t0}")
                            nc.tensor.matmul(p2r[:, :tn], blk2[:, k, 0], o1r[:, :tn], start=True, stop=False)
                            nc.tensor.matmul(p2r[:, :tn], blk2[:, k, 2], o1i[:, :tn], start=False, stop=True)
                            nc.tensor.matmul(p2i[:, :tn], blk2[:, k, 1], o1r[:, :tn], start=True, stop=False)
                            nc.tensor.matmul(p2i[:, :tn], blk2[:, k, 0], o1i[:, :tn], start=False, stop=True)
                            for comp, ps2 in ((0, p2r), (1, p2i)):
                                # softshrink(v) = relu(v-lam) - relu(-v-lam), v = ps2+b2
                                av = m2p.tile([BS, 512], FP, tag=f"av{comp}",
                                              name=f"av_{k}_{t0}_{comp}")
                                nc.scalar.activation(av[:, :tn], ps2[:BS, :tn], AF.Relu,
                                                     bias=b2m[:, comp, k:k + 1])
                                dv = m2p.tile([BS, 512], FP, tag=f"dv{comp}",
                                              name=f"dv_{k}_{t0}_{comp}")
                                nc.scalar.activation(dv[:, :tn], ps2[:BS, :tn], AF.Relu,
                                                     bias=b2pn[:, comp, k:k + 1],
                                                     scale=-1.0)
                                nc.vector.tensor_sub(zsb[:, k, comp, t0:t0 + tn],
                                                     av[:, :tn], dv[:, :tn])
                with (
                    tc.tile_pool(name="m3", bufs=2) as m3p,
                    tc.tile_pool(name="m3ps", bufs=2, space="PSUM") as m3ps,
                ):
                    # transpose back to [kh, c] then iFFT-H, then bounce out
                    for b in range(B):
                        for kwl in range(KWL):
                            tk0 = (b * KWL + kwl) * H
                            for k in range(NB):
                                po = m3ps.tile([H, 2, BS], BF, tag="po",
                                               name=f"po_{b}_{kwl}_{k}")
                                for comp in range(2):
                                    nc.tensor.transpose(
                                        po[:, comp, :], zsb[:, k, comp, tk0:tk0 + H],
                                        ident[0:BS, 0:BS])
                                if k % 2 == 0:
                                    nc.vector.tensor_copy(
                                        osb[:, b, kwl, :, k, :], po[:])
                                else:
                                    nc.scalar.copy(
                                        osb[:, b, kwl, :, k, :], po[:])
                            for si, (c0, cn) in enumerate(CS):
                                ks = si * 4
                                orr = osb[:, b, kwl, 0, ks:ks + 4, :]
                                ori = osb[:, b, kwl, 1, ks:ks + 4, :]
                                pur = m3ps.tile([128, 384], FP, tag="pur",
                                                name=f"pur_{b}_{kwl}_{si}")
                                pui = m3ps.tile([128, 384], FP, tag="pui",
                                                name=f"pui_{b}_{kwl}_{si}")
                                nc.tensor.matmul(pur[:], eir[:], orr, start=True, stop=False)
                                nc.tensor.matmul(pur[:], eiin[:], ori, start=False, stop=True)
                                nc.tensor.matmul(pui[:], eii[:], orr, start=True, stop=False)
                                nc.tensor.matmul(pui[:], eir[:], ori, start=False, stop=True)
                                ub = m3p.tile([H, 2, 384], BF, tag="ub",
                                              name=f"ub_{b}_{kwl}_{si}")
                                nc.scalar.copy(ub[:, 0], pur[:H])
                                nc.scalar.copy(ub[:, 1], pui[:H])
                                nc.sync.dma_start(
                                    out=v2in[b * H:(b + 1) * H, kwl, :, c0:c0 + cn],
                                    in_=ub[:])

            nc.gpsimd.collective_compute(
                "AllToAll", ALU.bypass, replica_groups=rg,
                ins=[a2a2_in[:]], outs=[a2a2_out[:]])

            # ========= STAGE B: iFFT-W + LN2 + MLP (interleaved) =========
            hv = h_dram.rearrange("r w c -> (r w) c")
            ov = out_sh.rearrange("r w c -> (r w) c")
            # MLP chunk ch needs B1 rows 0..req[ch]; emit rows LOOK ahead so
            # their Vector/Scalar work hides under the previous chunk's matmuls
            nch = (TOK + 511) // 512
            req = [-(-min((ch + 1) * 512, TOK) // W) - 1 for ch in range(nch)]
            LOOK = 3
            with (
                tc.tile_pool(name="sbB", bufs=1) as sbB,
                tc.tile_pool(name="b1p", bufs=2) as b1p,
                tc.tile_pool(name="b1ps", bufs=2, space="PSUM") as b1ps,
                tc.tile_pool(name="wpB", bufs=1) as wpB,
                tc.tile_pool(name="b2p", bufs=2) as b2p,
                tc.tile_pool(name="gp", bufs=1) as gp,
                tc.tile_pool(name="b2ps", bufs=2, space="PSUM") as b2ps,
                tc.tile_pool(name="b2psg", bufs=2, space="PSUM") as b2psg,
            ):
                mlp_in = sbB.tile([128, 6, TOK], BF)
                fc1w = wpB.tile([128, 6, 24, 128], BF)
                fc2w = wpB.tile([128, 24, 6, 128], BF)
                nc.sync.dma_start(out=fc1w[:], in_=fc1w_d.rearrange(
                    "(kc p) (m n) -> p kc m n", p=128, n=128))
                nc.sync.dma_start(out=fc2w[:], in_=fc2w_d.rearrange(
                    "(kc p) (m n) -> p kc m n", p=128, n=128))

                def mlp_chunk(ch):
                    t0 = ch * 512
                    tn = min(512, TOK - t0)
                    gsb = gp.tile([128, 24, 512], BF, tag="gsb", name=f"gsb_{t0}")
                    for m in range(24):
                        pg = b2psg.tile([128, 512], FP, tag="pg", name=f"pg_{t0}_{m}")
                        for kc in range(6):
                            nc.tensor.matmul(pg[:, :tn], fc1w[:, kc, m],
                                             mlp_in[:, kc, t0:t0 + tn],
                                             start=(kc == 0), stop=(kc == 5))
                        nc.scalar.activation(gsb[:, m, :tn], pg[:, :tn], AF.Gelu,
                                             bias=fc1b[:, m:m + 1])
                    fsb = []
                    for mo in range(6):
                        po = b2ps.tile([128, 512], FP, tag="pofc2",
                                       name=f"po_{t0}_{mo}")
                        for kc in range(24):
                            nc.tensor.matmul(po[:, :tn], fc2w[:, kc, mo],
                                             gsb[:, kc, :tn],
                                             start=(kc == 0), stop=(kc == 23))
                        fo = b2p.tile([128, 512], BF, tag=f"fo{mo}", bufs=1,
                                      name=f"fo_{t0}_{mo}")
                        nc.vector.tensor_scalar_add(fo[:, :tn], po[:, :tn],
                                                    fc2b[:, mo:mo + 1])
                        fsb.append(fo)
                    for ts0 in range(0, tn, 128):
                        tsn = min(128, tn - ts0)
                        hht = b2p.tile([128, C], BF, tag="hht", name=f"hht_{t0}_{ts0}")
                        nc.sync.dma_start(out=hht[:tsn],
                                          in_=hv[t0 + ts0:t0 + ts0 + tsn, :])
                        outt = b2p.tile([128, C], FP, tag="outt", name=f"outt_{t0}_{ts0}")
                        for mo in range(6):
                            ptt = b1ps.tile([128, 128], BF, tag="pt",
                                            name=f"ptt_{t0}_{ts0}_{mo}")
                            nc.tensor.transpose(ptt[:tsn, :],
                                                fsb[mo][:, ts0:ts0 + tsn],
                                                ident[:, :])
                            nc.vector.tensor_add(outt[:tsn, mo * 128:(mo + 1) * 128],
                                                 ptt[:tsn, :],
                                                 hht[:tsn, mo * 128:(mo + 1) * 128])
                        nc.sync.dma_start(out=ov[t0 + ts0:t0 + ts0 + tsn, :],
                                          in_=outt[:tsn])

                pend = []  # (row, h2, wn, tok0) awaiting transpose into mlp_in

                def flush_pt(upto):
                    keep = []
                    for rr, h2, wn_, tok0 in pend:
                        if rr > upto:
                            keep.append((rr, h2, wn_, tok0))
                            continue
                        for kc in range(6):
                            pt = b1ps.tile([128, 128], BF, tag="pt",
                                           name=f"pt_{tok0}_{kc}")
                            nc.tensor.transpose(
                                pt[:, :wn_],
                                h2[:wn_, kc // 3, (kc % 3) * 128:(kc % 3 + 1) * 128],
                                ident[:wn_, :wn_])
                            if kc % 2 == 0:
                                nc.vector.tensor_copy(mlp_in[:, kc, tok0:tok0 + wn_],
                                                      pt[:, :wn_])
                            else:
                                nc.scalar.copy(mlp_in[:, kc, tok0:tok0 + wn_],
                                               pt[:, :wn_])
                    pend[:] = keep

                def b1_row(r):
                    usb = b1p.tile([2 * KWP, C], BF, tag="usb", name=f"usb_{r}")
                    nc.sync.dma_start(out=usb[:], in_=a2a2_out[:, r])
                    for ci, (w0, wn) in enumerate(WCH):
                        stile = b1p.tile([128, 2, 384], BF, tag=f"stl{ci}", bufs=1,
                                         name=f"stl_{r}_{ci}")
                        nc.sync.dma_start(out=stile[:wn], in_=s_dram[r, w0:w0 + wn, :])
                        xb = b1p.tile([128, 2, 384], FP, tag=f"xb{ci}", bufs=1,
                                      name=f"xb_{r}_{ci}")
                        nc.sync.dma_start(out=xb[:wn], in_=x_sh[r, w0:w0 + wn, :])
                        sx = b1p.tile([128, 2, 384], BF, tag=f"sx{ci}",
                                      name=f"sx_{r}_{ci}")
                        nc.gpsimd.tensor_add(sx[:wn], stile[:wn], xb[:wn])
                        ht = b1p.tile([128, 2, 384], BF, tag=f"ht{ci}",
                                      name=f"ht_{r}_{ci}")
                        for si, (c0, cn) in enumerate(CS):
                            py = b1ps.tile([128, 384], FP, tag="py",
                                           name=f"py_{r}_{ci}_{si}")
                            nc.tensor.matmul(py[:wn], cw[:, w0:w0 + wn],
                                             usb[:, c0:c0 + cn], start=True, stop=True)
                            nc.vector.tensor_add(ht[:wn, si], py[:wn], sx[:wn, si])
                        nc.scalar.dma_start(out=h_dram[r, w0:w0 + wn, :], in_=ht[:wn])
                        # LN2 (weight/bias folded into fc1w/fc1b host-side)
                        st = b1p.tile([128, 2, 6], FP, tag=f"st{ci}", name=f"bst_{r}_{ci}")
                        nc.vector.bn_stats(st[:wn, 0, :], ht[:wn, 0])
                        nc.vector.bn_stats(st[:wn, 1, :], ht[:wn, 1])
                        ag = b1p.tile([128, 2], FP, tag=f"ag{ci}", name=f"bag_{r}_{ci}")
                        nc.vector.bn_aggr(ag[:wn], st[:wn])
                        sd = b1p.tile([128, 1], FP, tag=f"sd{ci}", name=f"bsd_{r}_{ci}")
                        nc.scalar.activation(sd[:wn], ag[:wn, 1:2], AF.Sqrt,
                                             bias=epsb[:wn])
                        rs = b1p.tile([128, 1], FP, tag=f"rs{ci}", name=f"brs_{r}_{ci}")
                        nc.vector.reciprocal(rs[:wn], sd[:wn])
                        h2 = b1p.tile([128, 2, 384], BF, tag=f"h2{ci}", bufs=6,
                                      name=f"bh2_{r}_{ci}")
                        nc.vector.tensor_scalar(h2[:wn], ht[:wn], ag[:wn, 0:1],
                                                rs[:wn], ALU.subtract, ALU.mult)
                        pend.append((r, h2, wn, r * W + w0))

                emitted = 0
                while emitted <= req[0]:
                    b1_row(emitted)
                    emitted += 1
                for ch in range(nch):
                    flush_pt(req[ch])
                    # rows for the NEXT chunk go before this chunk's matmuls so
                    # their Vector/Scalar work fills this chunk's Tensor shadow
                    nxt = req[ch + 1] if ch + 1 < nch else RPC - 1
                    while emitted <= nxt:
                        b1_row(emitted)
                        emitted += 1
                    mlp_chunk(ch)
    nc.compile()
    return nc


def _prep_inputs(inputs, fast1):
    consts = _dft_consts()
    x = np.asarray(inputs["x"], np.float32)
    xp = np.zeros((RPAD, W, C), np.float32)
    xp[:ROWS] = x.reshape(ROWS, W, C)
    w1 = np.asarray(inputs["w1"], np.float32)
    w2 = np.asarray(inputs["w2"], np.float32)
    blk1 = np.stack([w1[0], w1[1], -w1[1]], axis=1)  # [8,3,96,96]
    blk2 = np.stack([w2[0], w2[1], -w2[1]], axis=1)
    zp = np.zeros((NB, 3, BS, 128 - BS), np.float32)
    blk1 = np.concatenate([blk1, zp], axis=3).astype(bfloat16)
    blk2 = np.concatenate([blk2, zp], axis=3).astype(bfloat16)
    f32 = lambda k: np.ascontiguousarray(np.asarray(inputs[k], np.float32))
    b2 = f32("b2")
    # LN2 affine folded into fc1 (exact): y@((w*)fc1) + (b@fc1 + fc1_b)
    n2w = f32("norm2_w").reshape(C)
    n2b = f32("norm2_b").reshape(C)
    fc1w = f32("fc1_w")
    fc1w_eff = n2w[:, None] * fc1w
    fc1b_eff = f32("fc1_b").reshape(1, HID) + (n2b @ fc1w).reshape(1, HID)
    common = dict(
        fw_d=consts["fw"], ehr_d=consts["ehr"], ehi_d=consts["ehi"],
        ehin_d=consts["ehin"], eir_d=consts["eir"], eii_d=consts["eii"],
        eiin_d=consts["eiin"], cw_d=consts["cw"],
        blk1_d=blk1, blk2_d=blk2,
        b1_d=f32("b1"), b2m_d=np.ascontiguousarray(b2 - LAM),
        b2pn_d=np.ascontiguousarray(-(b2 + LAM)),
        fc1w_d=fc1w_eff.astype(bfloat16), fc2w_d=f32("fc2_w").astype(bfloat16),
        fc1b_d=np.ascontiguousarray(fc1b_eff), fc2b_d=f32("fc2_b").reshape(1, C),
    )
    if not fast1:
        common["n1w_d"] = f32("norm1_w").reshape(1, C)
        common["n1b_d"] = f32("norm1_b").reshape(1, C)
    in_maps = []
    for q in range(NCORES):
        m = dict(common)
        m["x_sh"] = np.ascontiguousarray(xp[q * RPC:(q + 1) * RPC])
        in_maps.append(m)
    return in_maps


last_exec_time_ns = None


def kernel(**inputs):
    global last_exec_time_ns
    bass_utils.upload_artifacts = lambda tmpdir: ""  # avoid bucket upload hang under trace
    fast1 = bool(
        np.allclose(np.asarray(inputs["norm1_w"], np.float32), 1.0)
        and np.allclose(np.asarray(inputs["norm1_b"], np.float32), 0.0))
    key = ("nc", fast1)
    if key not in _cache:
        _cache[key] = _build(fast1)
    nc = _cache[key]
    in_maps = _prep_inputs(inputs, fast1)
    _os = __import__("os")
    res = bass_utils.run_bass_kernel_spmd(
        nc, in_maps, core_ids=list(range(NCORES)),
        tmpdir=_os.environ.get("KERNEL_TRACE_DIR") or None,
        trace=bool(int(_os.environ.get("KERNEL_TRACE", "0"))))
    last_exec_time_ns = res.exec_time_ns
    out = np.concatenate([res.results[q]["out_sh"] for q in range(NCORES)], axis=0)
    return np.ascontiguousarray(out[:ROWS].reshape(B, H, W, C))


# revision 19
# speedup vs baseline: 1.4167x; 1.4167x over previous
"""AFNO transformer block (LN -> AFNO2D -> +res -> LN -> MLP -> +res) on 8 trn2 cores.

Distribution: spatial (b,h)-rows sharded 8x for LN1/FFT-W and iFFT-W/LN2/MLP stages;
kw-spectral-columns sharded 8x for FFT-H/block-MM/iFFT-H middle stage; two
AllToAlls (bf16 spectra) swap between the shardings. FFTs are dense matmuls
against DFT matrices (only 46 of 91 W-modes survive; all H modes kept).

v2: LN work spread across Scalar/GpSimd/Vector engines; LN2 folded into fc1
weights host-side; residuals stored bf16; softshrink via two Relu activations;
stage-B LN/iFFT interleaved with MLP chunks so Vector work hides under the
Tensor-bound matmuls.
"""
import sys

sys.path.insert(0, "/opt/trn_rl_repo")
import numpy as np
from ml_dtypes import bfloat16

from concourse import bacc, tile, mybir
from concourse import bass_utils
from concourse.masks import make_identity

FP = mybir.dt.float32
BF = mybir.dt.bfloat16
AF = mybir.ActivationFunctionType
ALU = mybir.AluOpType

NCORES = 8
B, H, W, C = 2, 90, 180, 768
NB, BS = 8, 96
KW, KWP = 46, 48
ROWS = B * H            # 180
RPAD = 192              # padded rows
RPC = RPAD // NCORES    # 24 rows per core
KWL = KWP // NCORES     # 6 kw per core
HID = 4 * C             # 3072
LN_EPS = 1e-5
LAM = 0.01              # softshrink lambda
TOK = RPC * W           # 4320 tokens per core in stage B
WCH = [(0, 128), (128, 52)]   # w-axis partition chunks
CS = [(0, 384), (384, 384)]   # channel free-dim slices

_cache = {}


def _dft_consts():
    wi, ki, hi = np.arange(W), np.arange(KWP), np.arange(H)
    aw = 2 * np.pi * np.outer(wi, ki) / W
    FWr = np.cos(aw) / np.sqrt(W)
    FWi = -np.sin(aw) / np.sqrt(W)
    FWr[:, KW:] = 0.0
    FWi[:, KW:] = 0.0
    fw = np.concatenate([FWr, FWi], axis=1)          # [180, 96]
    ah = 2 * np.pi * np.outer(hi, hi) / H
    ehr = np.cos(ah) / np.sqrt(H)                    # symmetric
    ehi = -np.sin(ah) / np.sqrt(H)
    ck = np.where(ki == 0, 1.0, 2.0)
    CWr = (ck[:, None] * np.cos(aw.T)) / np.sqrt(W)  # [48, 180]
    CWi = (-ck[:, None] * np.sin(aw.T)) / np.sqrt(W)
    CWi[0, :] = 0.0
    CWr[KW:, :] = 0.0
    CWi[KW:, :] = 0.0
    cw = np.concatenate([CWr, CWi], axis=0)          # [96, 180]
    # interleave real/imag so spectra rows come out in (kw, comp) order,
    # matching the a2a tensors' [s, kwl, comp] layout -> single-DMA moves
    il = np.array([[k, KWP + k] for k in range(KWP)]).reshape(-1)
    fw = fw[:, il]
    cw = cw[il, :]
    b16 = lambda a: np.ascontiguousarray(a).astype(bfloat16)
    padf = lambda a: np.concatenate([a, np.zeros((a.shape[0], 128 - a.shape[1]),
                                                 a.dtype)], axis=1)
    return dict(fw=b16(fw), ehr=b16(ehr), ehi=b16(ehi), ehin=b16(-ehi),
                eir=b16(padf(ehr)), eii=b16(padf(-ehi)), eiin=b16(padf(ehi)),
                cw=b16(cw))


def _build(fast1):
    nc = bacc.Bacc("TRN2", target_bir_lowering=False, debug=False,
                   num_devices=NCORES)

    def din(name, shape, dt=FP):
        return nc.dram_tensor(name, list(shape), dt, kind="ExternalInput").ap()

    x_sh = din("x_sh", [RPC, W, C])
    fw_d = din("fw_d", [W, 2 * KWP], BF)
    ehr_d = din("ehr_d", [H, H], BF)
    ehi_d = din("ehi_d", [H, H], BF)
    ehin_d = din("ehin_d", [H, H], BF)
    eir_d = din("eir_d", [H, 128], BF)
    eii_d = din("eii_d", [H, 128], BF)
    eiin_d = din("eiin_d", [H, 128], BF)
    cw_d = din("cw_d", [2 * KWP, W], BF)
    blk1_d = din("blk1_d", [NB, 3, BS, 128], BF)  # [w1r, w1i, -w1i], out-dim padded
    blk2_d = din("blk2_d", [NB, 3, BS, 128], BF)
    b1_d = din("b1_d", [2, NB, BS])
    b2m_d = din("b2m_d", [2, NB, BS])      # b2 - lam
    b2pn_d = din("b2pn_d", [2, NB, BS])    # -(b2 + lam)
    fc1w_d = din("fc1w_d", [C, HID], BF)   # LN2 weight folded in
    fc2w_d = din("fc2w_d", [HID, C], BF)
    fc1b_d = din("fc1b_d", [1, HID])       # LN2 bias folded in
    fc2b_d = din("fc2b_d", [1, C])
    if not fast1:
        n1w_d = din("n1w_d", [1, C])
        n1b_d = din("n1b_d", [1, C])

    out_sh = nc.dram_tensor("out_sh", [RPC, W, C], FP, kind="ExternalOutput").ap()

    s_dram = nc.dram_tensor("s_dram", [RPC, W, C], BF).ap()
    h_dram = nc.dram_tensor("h_dram", [RPC, W, C], BF).ap()
    a2a1_in = nc.dram_tensor("a2a1_in", [NCORES, RPC, KWL, 2, C], BF).ap()
    a2a1_out = nc.dram_tensor("a2a1_out", [NCORES, RPC, KWL, 2, C], BF).ap()
    a2a2_in = nc.dram_tensor("a2a2_in", [NCORES, RPC, KWL, 2, C], BF).ap()
    a2a2_out = nc.dram_tensor("a2a2_out", [NCORES, RPC, KWL, 2, C], BF).ap()

    rg = [list(range(NCORES))]

    with tile.TileContext(nc) as tc:
        with tc.tile_pool(name="cpool", bufs=1) as cp:
            # ---- small constants resident whole kernel ----
            ident = cp.tile([128, 128], BF)
            make_identity(nc, ident[:])
            fw_a = cp.tile([128, 2 * KWP], BF)
            fw_b = cp.tile([52, 2 * KWP], BF)
            nc.sync.dma_start(out=fw_a[:], in_=fw_d[0:128, :])
            nc.sync.dma_start(out=fw_b[:], in_=fw_d[128:180, :])
            ehr = cp.tile([H, H], BF); nc.sync.dma_start(out=ehr[:], in_=ehr_d[:])
            ehi = cp.tile([H, H], BF); nc.sync.dma_start(out=ehi[:], in_=ehi_d[:])
            ehin = cp.tile([H, H], BF); nc.sync.dma_start(out=ehin[:], in_=ehin_d[:])
            eir = cp.tile([H, 128], BF); nc.sync.dma_start(out=eir[:], in_=eir_d[:])
            eii = cp.tile([H, 128], BF); nc.sync.dma_start(out=eii[:], in_=eii_d[:])
            eiin = cp.tile([H, 128], BF); nc.sync.dma_start(out=eiin[:], in_=eiin_d[:])
            cw = cp.tile([2 * KWP, W], BF); nc.sync.dma_start(out=cw[:], in_=cw_d[:])
            b1c = cp.tile([BS, 2, NB], FP)
            b2m = cp.tile([BS, 2, NB], FP)
            b2pn = cp.tile([BS, 2, NB], FP)
            nc.sync.dma_start(out=b1c[:], in_=b1_d.rearrange("v k o -> o v k"))
            nc.sync.dma_start(out=b2m[:], in_=b2m_d.rearrange("v k o -> o v k"))
            nc.sync.dma_start(out=b2pn[:], in_=b2pn_d.rearrange("v k o -> o v k"))
            epsb = cp.tile([128, 1], FP)
            nc.vector.memset(epsb[:], LN_EPS)
            fc1b = cp.tile([128, 24], FP)
            fc2b = cp.tile([128, 6], FP)
            nc.sync.dma_start(out=fc1b[:], in_=fc1b_d.rearrange("x (m p) -> p (x m)", p=128))
            nc.sync.dma_start(out=fc2b[:], in_=fc2b_d.rearrange("x (m p) -> p (x m)", p=128))

            # ================= STAGE A: LN1 + FFT-W per row =================
            with (
                tc.tile_pool(name="sa", bufs=6) as sa,
                tc.tile_pool(name="na", bufs=1) as na,
                tc.tile_pool(name="saps", bufs=2, space="PSUM") as saps,
            ):
                if not fast1:
                    n1w_r = na.tile([128, 2, 384], FP)
                    n1b_r = na.tile([128, 2, 384], FP)
                    nc.sync.dma_start(out=n1w_r[:], in_=n1w_d[:].partition_broadcast(128))
                    nc.sync.dma_start(out=n1b_r[:], in_=n1b_d[:].partition_broadcast(128))
                for r in range(RPC):
                    h1bf = []
                    for ci, (w0, wn) in enumerate(WCH):
                        xa = sa.tile([128, 2, 384], FP, tag=f"xa{ci}", name=f"xa_{r}_{ci}")
                        nc.sync.dma_start(out=xa[:wn], in_=x_sh[r, w0:w0 + wn, :])
                        st = sa.tile([128, 2, 6], FP, tag=f"st{ci}", name=f"st_{r}_{ci}")
                        nc.vector.bn_stats(st[:wn, 0, :], xa[:wn, 0])
                        nc.vector.bn_stats(st[:wn, 1, :], xa[:wn, 1])
                        ag = sa.tile([128, 2], FP, tag=f"ag{ci}", name=f"ag_{r}_{ci}")
                        nc.vector.bn_aggr(ag[:wn], st[:wn])
                        sd = sa.tile([128, 1], FP, tag=f"sd{ci}", name=f"sd_{r}_{ci}")
                        nc.scalar.activation(sd[:wn], ag[:wn, 1:2], AF.Sqrt,
                                             bias=epsb[:wn])
                        rs = sa.tile([128, 1], FP, tag=f"rs{ci}", name=f"rs_{r}_{ci}")
                        nc.vector.reciprocal(rs[:wn], sd[:wn])
                        tb_ = sa.tile([128, 2, 384], BF, tag=f"tb{ci}", name=f"tb_{r}_{ci}")
                        nc.vector.tensor_scalar(tb_[:wn], xa[:wn], ag[:wn, 0:1],
                                                rs[:wn], ALU.subtract, ALU.mult)
                        if fast1:
                            hbf = tb_
                        else:
                            hw = sa.tile([128, 2, 384], BF, tag=f"hw{ci}", name=f"hw_{r}_{ci}")
                            nc.gpsimd.tensor_mul(hw[:wn], tb_[:wn], n1w_r[:wn])
                            hbf = sa.tile([128, 2, 384], BF, tag=f"hb{ci}", name=f"hb_{r}_{ci}")
                            nc.gpsimd.tensor_add(hbf[:wn], hw[:wn], n1b_r[:wn])
                        # s_dram holds LN1(x) only; stage B re-adds x
                        nc.scalar.dma_start(out=s_dram[r, w0:w0 + wn, :], in_=hbf[:wn])
                        h1bf.append(hbf)
                    for si, (c0, cn) in enumerate(CS):
                        psy = saps.tile([2 * KWP, 384], FP, tag="psy", name=f"psy_{r}_{si}")
                        nc.tensor.matmul(psy[:], fw_a[:], h1bf[0][:, si],
                                         start=True, stop=False)
                        nc.tensor.matmul(psy[:], fw_b[:], h1bf[1][0:52, si],
                                         start=False, stop=True)
                        yb = sa.tile([2 * KWP, 384], BF, tag="yb", name=f"yb_{r}_{si}")
                        nc.scalar.copy(yb[:], psy[:])
                        nc.sync.dma_start(out=a2a1_in[:, r, :, :, c0:c0 + cn],
                                          in_=yb[:])

            nc.gpsimd.collective_compute(
                "AllToAll", ALU.bypass, replica_groups=rg,
                ins=[a2a1_in[:]], outs=[a2a1_out[:]])

            # ================= MIDDLE: FFT-H, block MM, iFFT-H =================
            v1 = a2a1_out.rearrange("s r kwl comp c -> (s r) kwl comp c")
            v2in = a2a2_in.rearrange("s r kwl comp c -> (s r) kwl comp c")
            NTOK = B * KWL * H  # 1080 spectral tokens per core
            with tc.tile_pool(name="mz", bufs=1) as mz:
                zsb = mz.tile([BS, NB, 2, NTOK], BF)   # [96, k, comp, (b kwl kh)]
                osb = mz.tile([H, B, KWL, 2, NB, BS], BF)
                blk1 = mz.tile([BS, NB, 3, 128], BF)
                blk2 = mz.tile([BS, NB, 3, 128], BF)
                nc.sync.dma_start(out=blk1[:], in_=blk1_d.rearrange("k v i o -> i k v o"))
                nc.sync.dma_start(out=blk2[:], in_=blk2_d.rearrange("k v i o -> i k v o"))
                with (
                    tc.tile_pool(name="m1", bufs=2) as m1p,
                    tc.tile_pool(name="m1ps", bufs=2, space="PSUM") as m1ps,
                ):
                    for b in range(B):
                        ybt = m1p.tile([H, KWL, 2, C], BF, tag="ybt", name=f"ybt_{b}")
                        nc.sync.dma_start(out=ybt[:], in_=v1[b * H:(b + 1) * H])
                        for kwl in range(KWL):
                            for si, (c0, cn) in enumerate(CS):
                                pr = m1ps.tile([H, 384], FP, tag="pr", name=f"pr_{b}_{kwl}_{si}")
                                pi = m1ps.tile([H, 384], FP, tag="pi", name=f"pi_{b}_{kwl}_{si}")
                                rr = ybt[:, kwl, 0, c0:c0 + cn]
                                ri = ybt[:, kwl, 1, c0:c0 + cn]
                                nc.tensor.matmul(pr[:], ehr[:], rr, start=True, stop=False)
                                nc.tensor.matmul(pr[:], ehin[:], ri, start=False, stop=True)
                                nc.tensor.matmul(pi[:], ehi[:], rr, start=True, stop=False)
                                nc.tensor.matmul(pi[:], ehr[:], ri, start=False, stop=True)
                                tsb = m1p.tile([H, 2, 384], BF, tag="tsb",
                                               name=f"tsb_{b}_{kwl}_{si}")
                                nc.vector.tensor_copy(tsb[:, 0], pr[:])
                                nc.vector.tensor_copy(tsb[:, 1], pi[:])
                                for cb in range(4):
                                    k = si * 4 + cb
                                    pz = m1ps.tile([BS, 2, H], BF, tag="pz",
                                                   name=f"pz_{b}_{kwl}_{si}_{cb}")
                                    for comp in range(2):
                                        nc.tensor.transpose(
                                            pz[:, comp, :],
                                            tsb[:, comp, cb * BS:(cb + 1) * BS],
                                            ident[0:H, 0:H])
                                    tk0 = (b * KWL + kwl) * H
                                    if k % 2 == 0:
                                        nc.vector.tensor_copy(
                                            zsb[:, k, :, tk0:tk0 + H], pz[:])
                                    else:
                                        nc.scalar.copy(
                                            zsb[:, k, :, tk0:tk0 + H], pz[:])
                with (
                    tc.tile_pool(name="m2", bufs=2) as m2p,
                    tc.tile_pool(name="m2ps", bufs=2, space="PSUM") as m2ps,
                ):
                    BCH = [(0, 512), (512, 512), (1024, NTOK - 1024)]
                    for k in range(NB):
                        for t0, tn in BCH:
                            p1r = m2ps.tile([128, 512], FP, tag="p1r", name=f"p1r_{k}_{t0}")
                            p1i = m2ps.tile([128, 512], FP, tag="p1i", name=f"p1i_{k}_{t0}")
                            zr = zsb[:, k, 0, t0:t0 + tn]
                            zi = zsb[:, k, 1, t0:t0 + tn]
                            nc.tensor.matmul(p1r[:, :tn], blk1[:, k, 0], zr, start=True, stop=False)
                            nc.tensor.matmul(p1r[:, :tn], blk1[:, k, 2], zi, start=False, stop=True)
                            nc.tensor.matmul(p1i[:, :tn], blk1[:, k, 1], zr, start=True, stop=False)
                            nc.tensor.matmul(p1i[:, :tn], blk1[:, k, 0], zi, start=False, stop=True)
                            o1r = m2p.tile([BS, 512], BF, tag="o1r", name=f"o1r_{k}_{t0}")
                            o1i = m2p.tile([BS, 512], BF, tag="o1i", name=f"o1i_{k}_{t0}")
                            nc.scalar.activation(o1r[:, :tn], p1r[:BS, :tn], AF.Relu,
                                                 bias=b1c[:, 0, k:k + 1])
                            nc.scalar.activation(o1i[:, :tn], p1i[:BS, :tn], AF.Relu,
                                                 bias=b1c[:, 1, k:k + 1])
                            p2r = m2ps.tile([128, 512], FP, tag="p2r", name=f"p2r_{k}_{t0}")
                            p2i = m2ps.tile([128, 512], FP, tag="p2i", name=f"p2i_{k}_{t0}")
                            nc.tensor.matmul(p2r[:, :tn], blk2[:, k, 0], o1r[:, :tn], start=True, stop=False)
                            nc.tensor.matmul(p2r[:, :tn], blk2[:, k, 2], o1i[:, :tn], start=False, stop=True)
                            nc.tensor.matmul(p2i[:, :tn], blk2[:, k, 1], o1r[:, :tn], start=True, stop=False)
                            nc.tensor.matmul(p2i[:, :tn], blk2[:, k, 0], o1i[:, :tn], start=False, stop=True)
                            for comp, ps2 in ((0, p2r), (1, p2i)):
                                # softshrink(v) = relu(v-lam) - relu(-v-lam), v = ps2+b2
                                av = m2p.tile([BS, 512], FP, tag=f"av{comp}",
                                              name=f"av_{k}_{t0}_{comp}")
                                nc.scalar.activation(av[:, :tn], ps2[:BS, :tn], AF.Relu,
                                                     bias=b2m[:, comp, k:k + 1])
                                dv = m2p.tile([BS, 512], FP, tag=f"dv{comp}",
                                              name=f"dv_{k}_{t0}_{comp}")
                                nc.scalar.activation(dv[:, :tn], ps2[:BS, :tn], AF.Relu,
                                                     bias=b2pn[:, comp, k:k + 1],
                                                     scale=-1.0)
                                nc.vector.tensor_sub(zsb[:, k, comp, t0:t0 + tn],
                                                     av[:, :tn], dv[:, :tn])
                with (
                    tc.tile_pool(name="m3", bufs=2) as m3p,
                    tc.tile_pool(name="m3ps", bufs=2, space="PSUM") as m3ps,
                ):
                    # transpose back to [kh, c] then iFFT-H, then bounce out
                    for b in range(B):
                        for kwl in range(KWL):
                            tk0 = (b * KWL + kwl) * H
                            for k in range(NB):
                                po = m3ps.tile([H, 2, BS], BF, tag="po",
                                               name=f"po_{b}_{kwl}_{k}")
                                for comp in range(2):
                                    nc.tensor.transpose(
                                        po[:, comp, :], zsb[:, k, comp, tk0:tk0 + H],
                                        ident[0:BS, 0:BS])
                                if k % 2 == 0:
                                    nc.vector.tensor_copy(
                                        osb[:, b, kwl, :, k, :], po[:])
                                else:
                                    nc.scalar.copy(
                                        osb[:, b, kwl, :, k, :], po[:])
                            for si, (c0, cn) in enumerate(CS):
                                ks = si * 4
                                orr = osb[:, b, kwl, 0, ks:ks + 4, :]
                                ori = osb[:, b, kwl, 1, ks:ks + 4, :]
                                pur = m3ps.tile([128, 384], FP, tag="pur",
                                                name=f"pur_{b}_{kwl}_{si}")
                                pui = m3ps.tile([128, 384], FP, tag="pui",
                                                name=f"pui_{b}_{kwl}_{si}")
                                nc.tensor.matmul(pur[:], eir[:], orr, start=True, stop=False)
                                nc.tensor.matmul(pur[:], eiin[:], ori, start=False, stop=True)
                                nc.tensor.matmul(pui[:], eii[:], orr, start=True, stop=False)
                                nc.tensor.matmul(pui[:], eir[:], ori, start=False, stop=True)
                                ub = m3p.tile([H, 2, 384], BF, tag="ub",
                                              name=f"ub_{b}_{kwl}_{si}")
                                nc.scalar.copy(ub[:, 0], pur[:H])
                                nc.scalar.copy(ub[:, 1], pui[:H])
                                nc.sync.dma_start(
                                    out=v2in[b * H:(b + 1) * H, kwl, :, c0:c0 + cn],
                                    in_=ub[:])

            nc.gpsimd.collective_compute(
                "AllToAll", ALU.bypass, replica_groups=rg,
                ins=[a2a2_in[:]], outs=[a2a2_out[:]])

            # ========= STAGE B: iFFT-W + LN2 + MLP (interleaved) =========
            hv = h_dram.rearrange("r w c -> (r w) c")
            ov = out_sh.rearrange("r w c -> (r w) c")
            # MLP chunk ch needs B1 rows 0..req[ch]; emit rows LOOK ahead so
            # their Vector/Scalar work hides under the previous chunk's matmuls
            nch = (TOK + 511) // 512
            req = [-(-min((ch + 1) * 512, TOK) // W) - 1 for ch in range(nch)]
            LOOK = 3
            with (
                tc.tile_pool(name="sbB", bufs=1) as sbB,
                tc.tile_pool(name="b1p", bufs=2) as b1p,
                tc.tile_pool(name="b1ps", bufs=2, space="PSUM") as b1ps,
                tc.tile_pool(name="wpB", bufs=1) as wpB,
                tc.tile_pool(name="b2p", bufs=2) as b2p,
                tc.tile_pool(name="gp", bufs=1) as gp,
                tc.tile_pool(name="b2ps", bufs=2, space="PSUM") as b2ps,
                tc.tile_pool(name="b2psg", bufs=2, space="PSUM") as b2psg,
            ):
                mlp_in = sbB.tile([128, 6, TOK], BF)
                fc1w = wpB.tile([128, 6, 24, 128], BF)
                fc2w = wpB.tile([128, 24, 6, 128], BF)
                nc.sync.dma_start(out=fc1w[:], in_=fc1w_d.rearrange(
                    "(kc p) (m n) -> p kc m n", p=128, n=128))
                nc.sync.dma_start(out=fc2w[:], in_=fc2w_d.rearrange(
                    "(kc p) (m n) -> p kc m n", p=128, n=128))

                def mlp_chunk(ch):
                    t0 = ch * 512
                    tn = min(512, TOK - t0)
                    gsb = gp.tile([128, 24, 512], BF, tag="gsb", name=f"gsb_{t0}")
                    for m in range(24):
                        pg = b2psg.tile([128, 512], FP, tag="pg", name=f"pg_{t0}_{m}")
                        for kc in range(6):
                            nc.tensor.matmul(pg[:, :tn], fc1w[:, kc, m],
                                             mlp_in[:, kc, t0:t0 + tn],
                                             start=(kc == 0), stop=(kc == 5))
                        nc.scalar.activation(gsb[:, m, :tn], pg[:, :tn], AF.Gelu,
                                             bias=fc1b[:, m:m + 1])
                    fsb = []
                    for mo in range(6):
                        po = b2ps.tile([128, 512], FP, tag="pofc2",
                                       name=f"po_{t0}_{mo}")
                        for kc in range(24):
                            nc.tensor.matmul(po[:, :tn], fc2w[:, kc, mo],
                                             gsb[:, kc, :tn],
                                             start=(kc == 0), stop=(kc == 23))
                        fo = b2p.tile([128, 512], BF, tag=f"fo{mo}", bufs=1,
                                      name=f"fo_{t0}_{mo}")
                        nc.vector.tensor_scalar_add(fo[:, :tn], po[:, :tn],
                                                    fc2b[:, mo:mo + 1])
                        fsb.append(fo)
                    for ts0 in range(0, tn, 128):
                        tsn = min(128, tn - ts0)
                        hht = b2p.tile([128, C], BF, tag="hht", name=f"hht_{t0}_{ts0}")
                        nc.sync.dma_start(out=hht[:tsn],
                                          in_=hv[t0 + ts0:t0 + ts0 + tsn, :])
                        outt = b2p.tile([128, C], FP, tag="outt", name=f"outt_{t0}_{ts0}")
                        for mo in range(6):
                            ptt = b1ps.tile([128, 128], BF, tag="pt",
                                            name=f"ptt_{t0}_{ts0}_{mo}")
                            nc.tensor.transpose(ptt[:tsn, :],
                                                fsb[mo][:, ts0:ts0 + tsn],
                                                ident[:, :])
                            nc.vector.tensor_add(outt[:tsn, mo * 128:(mo + 1) * 128],
                                                 ptt[:tsn, :],
                                                 hht[:tsn, mo * 128:(mo + 1) * 128])
                        nc.sync.dma_start(out=ov[t0 + ts0:t0 + ts0 + tsn, :],
                                          in_=outt[:tsn])

                pend = []  # (row, h2, wn, tok0) awaiting transpose into mlp_in

                def flush_pt(upto):
                    keep = []
                    for rr, h2, wn_, tok0 in pend:
                        if rr > upto:
                            keep.append((rr, h2, wn_, tok0))
                            continue
                        for kc in range(6):
                            pt = b1ps.tile([128, 128], BF, tag="pt",
                                           name=f"pt_{tok0}_{kc}")
                            nc.tensor.transpose(
                                pt[:, :wn_],
                                h2[:wn_, kc // 3, (kc % 3) * 128:(kc % 3 + 1) * 128],
                                ident[:wn_, :wn_])
                            if kc % 2 == 0:
                                nc.vector.tensor_copy(mlp_in[:, kc, tok0:tok0 + wn_],
                                                      pt[:, :wn_])
                            else:
                                nc.scalar.copy(mlp_in[:, kc, tok0:tok0 + wn_],
                                               pt[:, :wn_])
                    pend[:] = keep

                def b1_row(r):
                    usb = b1p.tile([2 * KWP, C], BF, tag="usb", name=f"usb_{r}")
                    nc.sync.dma_start(out=usb[:], in_=a2a2_out[:, r])
                    for ci, (w0, wn) in enumerate(WCH):
                        stile = b1p.tile([128, 2, 384], BF, tag=f"stl{ci}", bufs=1,
                                         name=f"stl_{r}_{ci}")
                        nc.sync.dma_start(out=stile[:wn], in_=s_dram[r, w0:w0 + wn, :])
                        xb = b1p.tile([128, 2, 384], FP, tag=f"xb{ci}", bufs=1,
                                      name=f"xb_{r}_{ci}")
                        nc.sync.dma_start(out=xb[:wn], in_=x_sh[r, w0:w0 + wn, :])
                        sx = b1p.tile([128, 2, 384], BF, tag=f"sx{ci}",
                                      name=f"sx_{r}_{ci}")
                        nc.gpsimd.tensor_add(sx[:wn], stile[:wn], xb[:wn])
                        ht = b1p.tile([128, 2, 384], BF, tag=f"ht{ci}",
                                      name=f"ht_{r}_{ci}")
                        for si, (c0, cn) in enumerate(CS):
                            py = b1ps.tile([128, 384], FP, tag="py",
                                           name=f"py_{r}_{ci}_{si}")
                            nc.tensor.matmul(py[:wn], cw[:, w0:w0 + wn],
                                             usb[:, c0:c0 + cn], start=True, stop=True)
                            nc.vector.tensor_add(ht[:wn, si], py[:wn], sx[:wn, si])
                        nc.scalar.dma_start(out=h_dram[r, w0:w0 + wn, :], in_=ht[:wn])
                        # LN2 (weight/bias folded into fc1w/fc1b host-side)
                        st = b1p.tile([128, 2, 6], FP, tag=f"st{ci}", name=f"bst_{r}_{ci}")
                        nc.vector.bn_stats(st[:wn, 0, :], ht[:wn, 0])
                        nc.vector.bn_stats(st[:wn, 1, :], ht[:wn, 1])
                        ag = b1p.tile([128, 2], FP, tag=f"ag{ci}", name=f"bag_{r}_{ci}")
                        nc.vector.bn_aggr(ag[:wn], st[:wn])
                        sd = b1p.tile([128, 1], FP, tag=f"sd{ci}", name=f"bsd_{r}_{ci}")
                        nc.scalar.activation(sd[:wn], ag[:wn, 1:2], AF.Sqrt,
                                             bias=epsb[:wn])
                        rs = b1p.tile([128, 1], FP, tag=f"rs{ci}", name=f"brs_{r}_{ci}")
                        nc.vector.reciprocal(rs[:wn], sd[:wn])
                        h2 = b1p.tile([128, 2, 384], BF, tag=f"h2{ci}", bufs=6,
                                      name=f"bh2_{r}_{ci}")
                        nc.vector.tensor_scalar(h2[:wn], ht[:wn], ag[:wn, 0:1],
                                                rs[:wn], ALU.subtract, ALU.mult)
                        pend.append((r, h2, wn, r * W + w0))

                emitted = 0
                while emitted <= req[0]:
                    b1_row(emitted)
                    emitted += 1
                for ch in range(nch):
                    flush_pt(req[ch])
                    # rows for the NEXT chunk go before this chunk's matmuls so
                    # their Vector/Scalar work fills this chunk's Tensor shadow
                    nxt = req[ch + 1] if ch + 1 < nch else RPC - 1
                    while emitted <= nxt:
                        b1_row(emitted)
                        emitted += 1
                    mlp_chunk(ch)
    nc.compile()
    return nc


def _prep_inputs(inputs, fast1):
    consts = _dft_consts()
    x = np.asarray(inputs["x"], np.float32)
    xp = np.zeros((RPAD, W, C), np.float32)
    xp[:ROWS] = x.reshape(ROWS, W, C)
    w1 = np.asarray(inputs["w1"], np.float32)
    w2 = np.asarray(inputs["w2"], np.float32)
    blk1 = np.stack([w1[0], w1[1], -w1[1]], axis=1)  # [8,3,96,96]
    blk2 = np.stack([w2[0], w2[1], -w2[1]], axis=1)
    zp = np.zeros((NB, 3, BS, 128 - BS), np.float32)
    blk1 = np.concatenate([blk1, zp], axis=3).astype(bfloat16)
    blk2 = np.concatenate([blk2, zp], axis=3).astype(bfloat16)
    f32 = lambda k: np.ascontiguousarray(np.asarray(inputs[k], np.float32))
    b2 = f32("b2")
    # LN2 affine folded into fc1 (exact): y@((w*)fc1) + (b@fc1 + fc1_b)
    n2w = f32("norm2_w").reshape(C)
    n2b = f32("norm2_b").reshape(C)
    fc1w = f32("fc1_w")
    fc1w_eff = n2w[:, None] * fc1w
    fc1b_eff = f32("fc1_b").reshape(1, HID) + (n2b @ fc1w).reshape(1, HID)
    common = dict(
        fw_d=consts["fw"], ehr_d=consts["ehr"], ehi_d=consts["ehi"],
        ehin_d=consts["ehin"], eir_d=consts["eir"], eii_d=consts["eii"],
        eiin_d=consts["eiin"], cw_d=consts["cw"],
        blk1_d=blk1, blk2_d=blk2,
        b1_d=f32("b1"), b2m_d=np.ascontiguousarray(b2 - LAM),
        b2pn_d=np.ascontiguousarray(-(b2 + LAM)),
        fc1w_d=fc1w_eff.astype(bfloat16), fc2w_d=f32("fc2_w").astype(bfloat16),
        fc1b_d=np.ascontiguousarray(fc1b_eff), fc2b_d=f32("fc2_b").reshape(1, C),
    )
    if not fast1:
        common["n1w_d"] = f32("norm1_w").reshape(1, C)
        common["n1b_d"] = f32("norm1_b").reshape(1, C)
    in_maps = []
    for q in range(NCORES):
        m = dict(common)
        m["x_sh"] = np.ascontiguousarray(xp[q * RPC:(q + 1) * RPC])
        in_maps.append(m)
    return in_maps


last_exec_time_ns = None


def kernel(**inputs):
    global last_exec_time_ns
    bass_utils.upload_artifacts = lambda tmpdir: ""  # avoid bucket upload hang under trace
    fast1 = bool(
        np.allclose(np.asarray(inputs["norm1_w"], np.float32), 1.0)
        and np.allclose(np.asarray(inputs["norm1_b"], np.float32), 0.0))
    key = ("nc", fast1)
    if key not in _cache:
        _cache[key] = _build(fast1)
    nc = _cache[key]
    in_maps = _prep_inputs(inputs, fast1)
    _os = __import__("os")
    res = bass_utils.run_bass_kernel_spmd(
        nc, in_maps, core_ids=list(range(NCORES)),
        tmpdir=_os.environ.get("KERNEL_TRACE_DIR") or None,
        trace=bool(int(_os.environ.get("KERNEL_TRACE", "0"))))
    last_exec_time_ns = res.exec_time_ns
    out = np.concatenate([res.results[q]["out_sh"] for q in range(NCORES)], axis=0)
    return np.ascontiguousarray(out[:ROWS].reshape(B, H, W, C))
